# revision 19
# baseline (speedup 1.0000x reference)
"""Trainium2 Bass kernel for RoPE causal attention (B=2,S=2048,H=1024,nh=16).

Sharding: 8 cores = (batch b, head-group of 4). Each core computes
q/k/v projections for its 4 heads, RoPE, causal attention, and a partial
o-proj ( @ Wo rows for its heads).  Host sums the 4 partial y's per batch
and concatenates k/v head shards.
"""

import functools
import sys

import numpy as np

sys.path.insert(0, "/opt/trn_rl_repo")

import concourse.bass as bass  # noqa: E402
import concourse.mybir as mybir  # noqa: E402
from concourse import tile  # noqa: E402
from concourse.bass_utils import run_bass_kernel_spmd  # noqa: E402

B, S, H, NH, HD = 2, 2048, 1024, 16, 64
HPC = 4  # heads per core
DPC = HPC * HD  # 256 dims per core
NT = S // 128  # 16 s-tiles
F32 = mybir.dt.float32
F32R = mybir.dt.float32r
EXP = mybir.ActivationFunctionType.Exp


def r(ap):
    return ap.bitcast(F32R)


def bc(ap, n, axis=1):
    """Insert a broadcast (step 0) dim of size n at `axis` (after partition)."""
    raw = list(ap.ap)
    new = raw[:axis] + [[0, n]] + raw[axis:]
    return bass.AP(ap.tensor, ap.offset, new)


def _split_waits(nc, maxw=1):
    """This walrus build only accepts `maxw` sync-waits per instruction;
    hoist extras onto standalone event-semaphore waits (same engine,
    immediately before, preserving per-engine program order)."""
    for fn in nc.m.functions:
        for blk in fn.blocks:
            new = []
            for inst in blk.instructions:
                si = inst.sync_info
                if si is not None and si.on_wait is not None and len(si.on_wait) > maxw:
                    waits = list(si.on_wait)
                    for w in waits[:-maxw]:
                        new.append(
                            mybir.InstEventSemaphore(
                                name=nc.get_next_instruction_name(),
                                engine=inst.engine,
                                ins=[],
                                outs=[],
                                sync_info=mybir.SyncInfo(on_wait=[w], on_update=[]),
                            )
                        )
                    inst.sync_info = mybir.SyncInfo(
                        on_wait=waits[-maxw:], on_update=list(si.on_update)
                    )
                new.append(inst)
            blk.instructions = new


@functools.lru_cache(maxsize=1)
def build_nc():
    nc = bass.Bass()
    xT_d = nc.declare_dram_parameter("xT", [H, S], F32R, isOutput=False)
    wqT_d = nc.declare_dram_parameter("wqT", [H, DPC], F32R, isOutput=False)
    wkT_d = nc.declare_dram_parameter("wkT", [H, DPC], F32R, isOutput=False)
    wvT_d = nc.declare_dram_parameter("wvT", [H, DPC], F32R, isOutput=False)
    woT_d = nc.declare_dram_parameter("woT", [DPC, H], F32R, isOutput=False)
    cos_d = nc.declare_dram_parameter("cos", [S, HD], F32, isOutput=False)
    sinS_d = nc.declare_dram_parameter("sinS", [S, HD], F32, isOutput=False)
    mask_d = nc.declare_dram_parameter("maskT", [128, 128], F32R, isOutput=False)
    id_d = nc.declare_dram_parameter("ident", [128, 128], F32, isOutput=False)
    ones_d = nc.declare_dram_parameter("onesd", [128, 64], F32R, isOutput=False)
    m512_d = nc.declare_dram_parameter("mask512", [128, 4 * 512], F32R, isOutput=False)
    y_d = nc.declare_dram_parameter("y", [S, H], F32, isOutput=True)
    ko_d = nc.declare_dram_parameter("ko", [S, DPC], F32, isOutput=True)
    vo_d = nc.declare_dram_parameter("vo", [S, DPC], F32, isOutput=True)

    xT_r = xT_d[:].rearrange("(c p) s -> c p s", p=128)  # 8 x [128, S]
    y_r = y_d[:].rearrange("(t p) d -> p t d", p=128)
    ko_r = ko_d[:].rearrange("(t p) d -> p t d", p=128)
    vo_r = vo_d[:].rearrange("(t p) d -> p t d", p=128)

    with tile.TileContext(nc) as tc, nc.allow_low_precision(
        reason="fp32r (fp22-mantissa) matmul inputs; accumulation stays fp32"
    ):
        with (
            tc.tile_pool(name="persist", bufs=1) as pc,
            tc.tile_pool(name="psum_misc", bufs=1, space="PSUM") as pp_rb,
        ):
            # persistent tiles
            woT_sb = [pc.tile([128, H], F32R, tag=f"wo{j}", name=f"wo{j}") for j in range(2)]
            v65_sb = pc.tile([128, NT, HPC * 65], F32R, tag="v65", name="v65")
            qT_sb = [pc.tile([128, S], F32R, tag=f"qT{j}", name=f"qT{j}") for j in range(2)]
            kT_sb = [pc.tile([128, S], F32R, tag=f"kT{j}", name=f"kT{j}") for j in range(2)]
            outT_sb = [pc.tile([128, S], F32R, tag=f"oT{j}", name=f"oT{j}") for j in range(2)]
            mask_sb = pc.tile([128, 128], F32R, tag="mask", name="mask")
            id_sb = pc.tile([128, 128], F32, tag="id", name="id")
            ones_sb = pc.tile([1, 64], F32R, tag="ones", name="ones")

            m512_sb = pc.tile([128, 4, 512], F32R, tag="m512", name="m512")
            nc.sync.dma_start(
                m512_sb[:], m512_d[:].rearrange("p (d f) -> p d f", d=4)
            )
            nc.sync.dma_start(mask_sb[:], mask_d[:])
            nc.sync.dma_start(id_sb[:], id_d[:])
            for j in range(2):
                nc.sync.dma_start(
                    woT_sb[j][:], woT_d[j * 128 : (j + 1) * 128, :]
                )
            nc.sync.dma_start(ones_sb[:], ones_d[0:1, :])
            v65_4 = v65_sb[:].rearrange("p t (h e) -> p t h e", e=65)
            nc.sync.dma_start(
                v65_4[:, :, :, 64:65],
                ones_d[:].rearrange("p (t h one) -> p t h one", t=16, h=4),
            )

            # ---------------- Phase 1: projections + RoPE + transposes ----
            with (
                tc.tile_pool(name="ph1", bufs=3) as p1,
                tc.tile_pool(name="ph1x", bufs=2) as px,
                tc.tile_pool(name="ph1w", bufs=1) as p1w,
                tc.tile_pool(name="pp_proj", bufs=3, space="PSUM") as pp_proj,
                tc.tile_pool(name="pp_tr", bufs=2, space="PSUM") as pp_tr,
            ):
                wq_sb = p1w.tile([128, 8, DPC], F32R, tag="wq", name="wq")
                wk_sb = p1w.tile([128, 8, DPC], F32R, tag="wk", name="wk")
                wv_sb = p1w.tile([128, 8, DPC], F32R, tag="wv", name="wv")
                cos_sb = p1w.tile([128, NT, HD], F32, tag="cos", name="cos")
                sinS_sb = p1w.tile([128, NT, HD], F32, tag="sinS", name="sinS")
                q_sb = p1w.tile([128, NT, DPC], F32, tag="q", name="q")
                k_sb = p1w.tile([128, NT, DPC], F32, tag="k", name="k")
                for w_sb, w_d in zip((wq_sb, wk_sb, wv_sb), (wqT_d, wkT_d, wvT_d)):
                    nc.sync.dma_start(
                        w_sb[:], w_d[:].rearrange("(c p) d -> p c d", p=128)
                    )
                nc.sync.dma_start(
                    cos_sb[:], cos_d[:].rearrange("(t p) d -> p t d", p=128)
                )
                nc.sync.dma_start(
                    sinS_sb[:], sinS_d[:].rearrange("(t p) d -> p t d", p=128)
                )

                xT_p = xT_d[:].rearrange("(c p) s -> p c s", p=128)
                for half in range(4):
                    xh = px.tile([128, 8, S // 4], F32R, tag="xh", name="xh")
                    nc.sync.dma_start(
                        xh[:],
                        xT_p[:, :, half * (S // 4) : (half + 1) * (S // 4)],
                    )
                    for tt in range(half * (NT // 4), (half + 1) * (NT // 4)):
                        tl = (tt % (NT // 4)) * 128
                        for which, w_sb in (("q", wq_sb), ("k", wk_sb), ("v", wv_sb)):
                            ps = pp_proj.tile([128, DPC], F32, tag="proj", name="proj")
                            for c in range(8):
                                nc.tensor.matmul(
                                    ps[:],
                                    xh[:, c, tl : tl + 128],
                                    w_sb[:, c, :],
                                    start=(c == 0),
                                    stop=(c == 7),
                                )
                            if which == "v":
                                nc.vector.tensor_copy(
                                    v65_4[:, tt, :, 0:64],
                                    ps[:].rearrange("p (h e) -> p h e", e=64),
                                )
                            else:
                                dst = q_sb if which == "q" else k_sb
                                ps4 = ps[:].rearrange(
                                    "p (h two d) -> p h two d", two=2, d=32
                                )
                                cs = bc(cos_sb[:, tt, :], HPC)  # [128,4,64]
                                sn = bc(
                                    sinS_sb[:, tt, :].rearrange(
                                        "p (two d) -> p two d", two=2
                                    ),
                                    HPC,
                                )  # [128,4,2,32]
                                tmp = p1.tile([128, DPC], F32, tag="tmp", name="tmp")
                                tmp4 = tmp[:].rearrange(
                                    "p (h two d) -> p h two d", two=2, d=32
                                )
                                tmp2 = p1.tile([128, DPC], F32, tag="tmp2", name="tmp2")
                                nc.vector.tensor_mul(
                                    tmp4[:, :, 0, :], ps4[:, :, 1, :], sn[:, :, 0, :]
                                )
                                nc.vector.tensor_mul(
                                    tmp4[:, :, 1, :], ps4[:, :, 0, :], sn[:, :, 1, :]
                                )
                                nc.vector.tensor_mul(
                                    tmp2[:].rearrange("p (h d) -> p h d", d=HD),
                                    ps[:].rearrange("p (h d) -> p h d", d=HD),
                                    cs,
                                )
                                nc.vector.tensor_add(
                                    dst[:, tt, :], tmp[:], tmp2[:]
                                )
                # k/v outputs
                nc.sync.dma_start(ko_r[:, :, :], k_sb[:])
                for h in range(HPC):
                    nc.sync.dma_start(
                        vo_r[:, :, h * 64 : (h + 1) * 64],
                        v65_4[:, :, h, 0:64].bitcast(F32),
                    )
                # transposes q,k -> qT,kT
                for src, dstT in ((q_sb, qT_sb), (k_sb, kT_sb)):
                    for tt in range(NT):
                        for j in range(2):
                            pt = pp_tr.tile([128, 128], F32, tag="tr", name="tr")
                            nc.tensor.transpose(
                                pt[:], src[:, tt, j * 128 : (j + 1) * 128], id_sb[:]
                            )
                            if (tt + j) % 2 == 0:
                                nc.scalar.copy(
                                    dstT[j][:, tt * 128 : (tt + 1) * 128], pt[:]
                                )
                            else:
                                nc.vector.tensor_copy(
                                    dstT[j][:, tt * 128 : (tt + 1) * 128], pt[:]
                                )

            # ---------------- Phase 2: attention ------------------------
            with (
                tc.tile_pool(name="att", bufs=4) as pa,
                tc.tile_pool(name="attn_n", bufs=2) as pn,
                tc.tile_pool(name="pp_sc", bufs=2, space="PSUM") as pp_sc,
                tc.tile_pool(name="pp_av", bufs=2, space="PSUM") as pp_av,
            ):
                for h in range(HPC):
                    j, po = h // 2, (h % 2) * 64
                    kTh = kT_sb[j]
                    qTh = qT_sb[j]
                    for qc in range(4):
                        psav = pp_av.tile([65, 512], F32, tag="av", name="av")
                        nkb = 4 * qc + 4
                        for kb0 in range(0, nkb, 2):
                            psc = pp_sc.tile([128, 2, 512], F32, tag="sc", name="sc")
                            for i in range(2):
                                kb = kb0 + i
                                nc.tensor.matmul(
                                    psc[:, i, :],
                                    kTh[po : po + 64, kb * 128 : (kb + 1) * 128],
                                    qTh[po : po + 64, qc * 512 : (qc + 1) * 512],
                                    start=True,
                                    stop=True,
                                )
                            at = pa.tile([128, 2, 512], F32R, tag="attn", name="attn")
                            nc.scalar.activation(at[:], psc[:], EXP, scale=0.125)
                            for i in range(2):
                                kb = kb0 + i
                                if kb >= nkb - 4:
                                    d = kb - (nkb - 4)
                                    nc.vector.tensor_mul(
                                        at[:, i, :], at[:, i, :], m512_sb[:, d, :]
                                    )
                            for i in range(2):
                                kb = kb0 + i
                                nc.tensor.matmul(
                                    psav[:],
                                    v65_sb[:, kb, h * 65 : h * 65 + 65],
                                    at[:, i, :],
                                    start=(kb == 0),
                                    stop=(kb == nkb - 1),
                                )
                        # normalize: outT = psav[0:64] * (1/psav[64])
                        rec = pn.tile([1, 512], F32R, tag="rec", name="rec")
                        nc.vector.reciprocal(rec[:], psav[64:65, :])
                        ov = pn.tile([64, 512], F32R, tag="ov", name="ov")
                        nc.scalar.copy(ov[:], psav[0:64, :])
                        prb = pp_rb.tile([64, 512], F32, tag="rb", name="rb")
                        nc.tensor.matmul(
                            prb[:], ones_sb[:], rec[:], start=True, stop=True
                        )
                        nc.vector.tensor_mul(
                            outT_sb[j][po : po + 64, qc * 512 : (qc + 1) * 512],
                            ov[:],
                            prb[:],
                        )

            # ---------------- Phase 3: o-proj ---------------------------
            with (
                tc.tile_pool(name="ph3", bufs=1) as p3,
                tc.tile_pool(name="pp_y", bufs=2, space="PSUM") as pp_y,
            ):
                for st in range(NT):
                    py = pp_y.tile([128, H], F32, tag="y", name="y")
                    for nb in range(2):
                        for j in range(2):
                            nc.tensor.matmul(
                                py[:, nb * 512 : (nb + 1) * 512],
                                outT_sb[j][:, st * 128 : (st + 1) * 128],
                                woT_sb[j][:, nb * 512 : (nb + 1) * 512],
                                start=(j == 0),
                                stop=(j == 1),
                            )
                    ysb = p3.tile([128, H], F32, tag="ysb", name="ysb", bufs=3)
                    if st % 2 == 0:
                        nc.scalar.copy(ysb[:], py[:])
                    else:
                        nc.vector.tensor_copy(ysb[:], py[:])
                    nc.sync.dma_start(y_r[:, st, :], ysb[:])

    _split_waits(nc)
    return nc


def _rope_tables(position_ids):
    inv = (
        1.0
        / (10000.0 ** (np.arange(0, HD, 2, dtype=np.float32) / np.float32(HD)))
    ).astype(np.float32)
    t = np.arange(S, dtype=np.float32)
    freqs = (t[:, None] * inv[None, :]).astype(np.float32)
    emb = np.concatenate([freqs, freqs], axis=-1)
    cosf = np.cos(emb).astype(np.float32)
    sinf = np.sin(emb).astype(np.float32)
    pos = np.asarray(position_ids)
    cos_b = cosf[pos]  # [B, S, HD]
    sin_b = sinf[pos]
    sinS = sin_b.copy()
    sinS[:, :, : HD // 2] *= -1.0
    return cos_b, sinS


def kernel(x, position_ids, Wq, Wk, Wv, Wo):
    x = np.asarray(x, dtype=np.float32)
    Wq, Wk, Wv, Wo = (np.asarray(w, dtype=np.float32) for w in (Wq, Wk, Wv, Wo))
    cos_b, sinS_b = _rope_tables(position_ids)
    maskT = np.triu(np.ones((128, 128), dtype=np.float32))  # keep k<=q
    blocks = []
    for d in range(4):
        row = [np.zeros((128, 128), np.float32)] * d + [maskT] + [
            np.ones((128, 128), np.float32)
        ] * (3 - d)
        blocks.append(np.concatenate(row, axis=1))
    mask512 = np.concatenate(blocks, axis=1)  # [128, 4*512]
    ident = np.eye(128, dtype=np.float32)

    in_maps = []
    for c in range(8):
        b, hg = c // 4, (c % 4) * HPC
        rs = slice(hg * HD, hg * HD + DPC)
        in_maps.append(
            {
                "xT": np.ascontiguousarray(x[b].T),
                "wqT": np.ascontiguousarray(Wq[rs, :].T),
                "wkT": np.ascontiguousarray(Wk[rs, :].T),
                "wvT": np.ascontiguousarray(Wv[rs, :].T),
                "woT": np.ascontiguousarray(Wo[:, rs].T),
                "cos": np.ascontiguousarray(cos_b[b]),
                "sinS": np.ascontiguousarray(sinS_b[b]),
                "maskT": maskT,
                "ident": ident,
                "onesd": np.ones((128, 64), dtype=np.float32),
                "mask512": mask512,
            }
        )

    nc = build_nc()
    res = run_bass_kernel_spmd(nc, in_maps, list(range(8))).results

    y = np.zeros((B, S, H), dtype=np.float32)
    k = np.zeros((B, NH, S, HD), dtype=np.float32)
    v = np.zeros((B, NH, S, HD), dtype=np.float32)
    for c in range(8):
        b, hg = c // 4, (c % 4) * HPC
        y[b] += res[c]["y"]
        k[b, hg : hg + HPC] = (
            res[c]["ko"].reshape(S, HPC, HD).transpose(1, 0, 2)
        )
        v[b, hg : hg + HPC] = (
            res[c]["vo"].reshape(S, HPC, HD).transpose(1, 0, 2)
        )
    return y, k, v


# revision 24
# speedup vs baseline: 1.0036x; 1.0036x over previous
"""Trainium2 Bass kernel for RoPE causal attention (B=2,S=2048,H=1024,nh=16).

Sharding: 8 cores = (batch b, head-group of 4). Each core computes
q/k/v projections for its 4 heads, RoPE, causal attention, and a partial
o-proj ( @ Wo rows for its heads).  Host sums the 4 partial y's per batch
and concatenates k/v head shards.
"""

import functools
import sys

import numpy as np

sys.path.insert(0, "/opt/trn_rl_repo")

import concourse.bass as bass  # noqa: E402
import concourse.mybir as mybir  # noqa: E402
from concourse import tile  # noqa: E402
from concourse.bass_utils import run_bass_kernel_spmd  # noqa: E402

B, S, H, NH, HD = 2, 2048, 1024, 16, 64
HPC = 4  # heads per core
DPC = HPC * HD  # 256 dims per core
NT = S // 128  # 16 s-tiles
F32 = mybir.dt.float32
F32R = mybir.dt.float32r
EXP = mybir.ActivationFunctionType.Exp


def r(ap):
    return ap.bitcast(F32R)


def bc(ap, n, axis=1):
    """Insert a broadcast (step 0) dim of size n at `axis` (after partition)."""
    raw = list(ap.ap)
    new = raw[:axis] + [[0, n]] + raw[axis:]
    return bass.AP(ap.tensor, ap.offset, new)


def _split_waits(nc, maxw=1):
    """This walrus build only accepts `maxw` sync-waits per instruction;
    hoist extras onto standalone event-semaphore waits (same engine,
    immediately before, preserving per-engine program order)."""
    for fn in nc.m.functions:
        for blk in fn.blocks:
            new = []
            for inst in blk.instructions:
                si = inst.sync_info
                if si is not None and si.on_wait is not None and len(si.on_wait) > maxw:
                    waits = list(si.on_wait)
                    for w in waits[:-maxw]:
                        new.append(
                            mybir.InstEventSemaphore(
                                name=nc.get_next_instruction_name(),
                                engine=inst.engine,
                                ins=[],
                                outs=[],
                                sync_info=mybir.SyncInfo(on_wait=[w], on_update=[]),
                            )
                        )
                    inst.sync_info = mybir.SyncInfo(
                        on_wait=waits[-maxw:], on_update=list(si.on_update)
                    )
                new.append(inst)
            blk.instructions = new


@functools.lru_cache(maxsize=1)
def build_nc():
    nc = bass.Bass()
    xT_d = nc.declare_dram_parameter("xT", [H, S], F32R, isOutput=False)
    wqT_d = nc.declare_dram_parameter("wqT", [H, DPC], F32R, isOutput=False)
    wkT_d = nc.declare_dram_parameter("wkT", [H, DPC], F32R, isOutput=False)
    wvT_d = nc.declare_dram_parameter("wvT", [H, DPC], F32R, isOutput=False)
    woT_d = nc.declare_dram_parameter("woT", [DPC, H], F32R, isOutput=False)
    cos_d = nc.declare_dram_parameter("cos", [S, HD], F32, isOutput=False)
    sinS_d = nc.declare_dram_parameter("sinS", [S, HD], F32, isOutput=False)
    mask_d = nc.declare_dram_parameter("maskT", [128, 128], F32R, isOutput=False)
    id_d = nc.declare_dram_parameter("ident", [128, 128], F32, isOutput=False)
    ones_d = nc.declare_dram_parameter("onesd", [128, 64], F32R, isOutput=False)
    m512_d = nc.declare_dram_parameter("mask512", [128, 4 * 512], F32R, isOutput=False)
    y_d = nc.declare_dram_parameter("y", [S, H], F32, isOutput=True)
    ko_d = nc.declare_dram_parameter("ko", [S, DPC], F32, isOutput=True)
    vo_d = nc.declare_dram_parameter("vo", [S, DPC], F32, isOutput=True)

    xT_r = xT_d[:].rearrange("(c p) s -> c p s", p=128)  # 8 x [128, S]
    y_r = y_d[:].rearrange("(t p) d -> p t d", p=128)
    ko_r = ko_d[:].rearrange("(t p) d -> p t d", p=128)
    vo_r = vo_d[:].rearrange("(t p) d -> p t d", p=128)

    with tile.TileContext(nc) as tc, nc.allow_low_precision(
        reason="fp32r (fp22-mantissa) matmul inputs; accumulation stays fp32"
    ):
        with (
            tc.tile_pool(name="persist", bufs=1) as pc,
            tc.tile_pool(name="psum_misc", bufs=1, space="PSUM") as pp_rb,
        ):
            # persistent tiles
            woT_sb = [pc.tile([128, H], F32R, tag=f"wo{j}", name=f"wo{j}") for j in range(2)]
            v65_sb = pc.tile([128, NT, HPC * 65], F32R, tag="v65", name="v65")
            qT_sb = [pc.tile([128, S], F32R, tag=f"qT{j}", name=f"qT{j}") for j in range(2)]
            kT_sb = [pc.tile([128, S], F32R, tag=f"kT{j}", name=f"kT{j}") for j in range(2)]
            outT_sb = [pc.tile([128, S], F32R, tag=f"oT{j}", name=f"oT{j}") for j in range(2)]
            mask_sb = pc.tile([128, 128], F32R, tag="mask", name="mask")
            id_sb = pc.tile([128, 128], F32, tag="id", name="id")
            ones_sb = pc.tile([1, 64], F32R, tag="ones", name="ones")

            m512_sb = pc.tile([128, 4, 512], F32R, tag="m512", name="m512")
            nc.sync.dma_start(
                m512_sb[:], m512_d[:].rearrange("p (d f) -> p d f", d=4)
            )
            nc.sync.dma_start(mask_sb[:], mask_d[:])
            nc.sync.dma_start(id_sb[:], id_d[:])
            for j in range(2):
                nc.sync.dma_start(
                    woT_sb[j][:], woT_d[j * 128 : (j + 1) * 128, :]
                )
            nc.sync.dma_start(ones_sb[:], ones_d[0:1, :])
            v65_4 = v65_sb[:].rearrange("p t (h e) -> p t h e", e=65)
            nc.sync.dma_start(
                v65_4[:, :, :, 64:65],
                ones_d[:].rearrange("p (t h one) -> p t h one", t=16, h=4),
            )

            # ---------------- Phase 1: projections + RoPE + transposes ----
            with (
                tc.tile_pool(name="ph1", bufs=3) as p1,
                tc.tile_pool(name="ph1x", bufs=2) as px,
                tc.tile_pool(name="ph1w", bufs=1) as p1w,
                tc.tile_pool(name="pp_proj", bufs=3, space="PSUM") as pp_proj,
                tc.tile_pool(name="pp_tr", bufs=2, space="PSUM") as pp_tr,
            ):
                wq_sb = p1w.tile([128, 8, DPC], F32R, tag="wq", name="wq")
                wk_sb = p1w.tile([128, 8, DPC], F32R, tag="wk", name="wk")
                wv_sb = p1w.tile([128, 8, DPC], F32R, tag="wv", name="wv")
                cos_sb = p1w.tile([128, NT, HD], F32, tag="cos", name="cos")
                sinS_sb = p1w.tile([128, NT, HD], F32, tag="sinS", name="sinS")
                q_sb = p1w.tile([128, NT, DPC], F32, tag="q", name="q")
                k_sb = p1w.tile([128, NT, DPC], F32, tag="k", name="k")
                for w_sb, w_d in zip((wq_sb, wk_sb, wv_sb), (wqT_d, wkT_d, wvT_d)):
                    nc.sync.dma_start(
                        w_sb[:], w_d[:].rearrange("(c p) d -> p c d", p=128)
                    )
                nc.sync.dma_start(
                    cos_sb[:], cos_d[:].rearrange("(t p) d -> p t d", p=128)
                )
                nc.sync.dma_start(
                    sinS_sb[:], sinS_d[:].rearrange("(t p) d -> p t d", p=128)
                )

                xT_p = xT_d[:].rearrange("(c p) s -> p c s", p=128)
                for half in range(4):
                    xh = px.tile([128, 8, S // 4], F32R, tag="xh", name="xh")
                    nc.sync.dma_start(
                        xh[:],
                        xT_p[:, :, half * (S // 4) : (half + 1) * (S // 4)],
                    )
                    for tt in range(half * (NT // 4), (half + 1) * (NT // 4)):
                        tl = (tt % (NT // 4)) * 128
                        for which, w_sb in (("q", wq_sb), ("k", wk_sb), ("v", wv_sb)):
                            ps = pp_proj.tile([128, DPC], F32, tag="proj", name="proj")
                            for c in range(8):
                                nc.tensor.matmul(
                                    ps[:],
                                    xh[:, c, tl : tl + 128],
                                    w_sb[:, c, :],
                                    start=(c == 0),
                                    stop=(c == 7),
                                )
                            if which == "v":
                                nc.vector.tensor_copy(
                                    v65_4[:, tt, :, 0:64],
                                    ps[:].rearrange("p (h e) -> p h e", e=64),
                                )
                            else:
                                dst = q_sb if which == "q" else k_sb
                                ps4 = ps[:].rearrange(
                                    "p (h two d) -> p h two d", two=2, d=32
                                )
                                cs = bc(cos_sb[:, tt, :], HPC)  # [128,4,64]
                                sn = bc(
                                    sinS_sb[:, tt, :].rearrange(
                                        "p (two d) -> p two d", two=2
                                    ),
                                    HPC,
                                )  # [128,4,2,32]
                                tmp = p1.tile([128, DPC], F32, tag="tmp", name="tmp")
                                tmp4 = tmp[:].rearrange(
                                    "p (h two d) -> p h two d", two=2, d=32
                                )
                                tmp2 = p1.tile([128, DPC], F32, tag="tmp2", name="tmp2")
                                nc.vector.tensor_mul(
                                    tmp4[:, :, 0, :], ps4[:, :, 1, :], sn[:, :, 0, :]
                                )
                                nc.vector.tensor_mul(
                                    tmp4[:, :, 1, :], ps4[:, :, 0, :], sn[:, :, 1, :]
                                )
                                nc.vector.tensor_mul(
                                    tmp2[:].rearrange("p (h d) -> p h d", d=HD),
                                    ps[:].rearrange("p (h d) -> p h d", d=HD),
                                    cs,
                                )
                                nc.vector.tensor_add(
                                    dst[:, tt, :], tmp[:], tmp2[:]
                                )
                # k/v outputs
                nc.sync.dma_start(ko_r[:, :, :], k_sb[:])
                for h in range(HPC):
                    nc.sync.dma_start(
                        vo_r[:, :, h * 64 : (h + 1) * 64],
                        v65_4[:, :, h, 0:64].bitcast(F32),
                    )
                # transposes q,k -> qT,kT
                for src, dstT in ((q_sb, qT_sb), (k_sb, kT_sb)):
                    for tt in range(NT):
                        for j in range(2):
                            pt = pp_tr.tile([128, 128], F32, tag="tr", name="tr")
                            nc.tensor.transpose(
                                pt[:], src[:, tt, j * 128 : (j + 1) * 128], id_sb[:]
                            )
                            if (tt + j) % 2 == 0:
                                nc.scalar.copy(
                                    dstT[j][:, tt * 128 : (tt + 1) * 128], pt[:]
                                )
                            else:
                                nc.vector.tensor_copy(
                                    dstT[j][:, tt * 128 : (tt + 1) * 128], pt[:]
                                )

            # ---------------- Phase 2: attention ------------------------
            with (
                tc.tile_pool(name="att", bufs=6) as pa,
                tc.tile_pool(name="attn_n", bufs=3) as pn,
                tc.tile_pool(name="pp_sc", bufs=2, space="PSUM") as pp_sc,
                tc.tile_pool(name="pp_av", bufs=2, space="PSUM") as pp_av,
            ):
                for h in range(HPC):
                    j, po = h // 2, (h % 2) * 64
                    kTh = kT_sb[j]
                    qTh = qT_sb[j]
                    for qc in range(4):
                        psav = pp_av.tile([65, 512], F32, tag="av", name="av")
                        nkb = 4 * qc + 4
                        for kb0 in range(0, nkb, 2):
                            psc = pp_sc.tile([128, 2, 512], F32, tag="sc", name="sc")
                            for i in range(2):
                                kb = kb0 + i
                                nc.tensor.matmul(
                                    psc[:, i, :],
                                    kTh[po : po + 64, kb * 128 : (kb + 1) * 128],
                                    qTh[po : po + 64, qc * 512 : (qc + 1) * 512],
                                    start=True,
                                    stop=True,
                                )
                            at = pa.tile([128, 2, 512], F32R, tag="attn", name="attn")
                            nc.scalar.activation(at[:], psc[:], EXP, scale=0.125)
                            for i in range(2):
                                kb = kb0 + i
                                if kb >= nkb - 4:
                                    d = kb - (nkb - 4)
                                    nc.vector.tensor_mul(
                                        at[:, i, :], at[:, i, :], m512_sb[:, d, :]
                                    )
                            for i in range(2):
                                kb = kb0 + i
                                nc.tensor.matmul(
                                    psav[:],
                                    v65_sb[:, kb, h * 65 : h * 65 + 65],
                                    at[:, i, :],
                                    start=(kb == 0),
                                    stop=(kb == nkb - 1),
                                )
                        # normalize: outT = psav[0:64] * (1/psav[64])
                        rec = pn.tile([1, 512], F32R, tag="rec", name="rec")
                        nc.vector.reciprocal(rec[:], psav[64:65, :])
                        ov = pn.tile([64, 512], F32R, tag="ov", name="ov")
                        nc.scalar.copy(ov[:], psav[0:64, :])
                        prb = pp_rb.tile([64, 512], F32, tag="rb", name="rb")
                        nc.tensor.matmul(
                            prb[:], ones_sb[:], rec[:], start=True, stop=True
                        )
                        nc.vector.tensor_mul(
                            outT_sb[j][po : po + 64, qc * 512 : (qc + 1) * 512],
                            ov[:],
                            prb[:],
                        )

            # ---------------- Phase 3: o-proj ---------------------------
            with (
                tc.tile_pool(name="ph3", bufs=1) as p3,
                tc.tile_pool(name="pp_y", bufs=2, space="PSUM") as pp_y,
            ):
                for st in range(NT):
                    py = pp_y.tile([128, H], F32, tag="y", name="y")
                    for nb in range(2):
                        for j in range(2):
                            nc.tensor.matmul(
                                py[:, nb * 512 : (nb + 1) * 512],
                                outT_sb[j][:, st * 128 : (st + 1) * 128],
                                woT_sb[j][:, nb * 512 : (nb + 1) * 512],
                                start=(j == 0),
                                stop=(j == 1),
                            )
                    ysb = p3.tile([128, H], F32, tag="ysb", name="ysb", bufs=3)
                    if st % 2 == 0:
                        nc.scalar.copy(ysb[:], py[:])
                    else:
                        nc.vector.tensor_copy(ysb[:], py[:])
                    nc.sync.dma_start(y_r[:, st, :], ysb[:])

    _split_waits(nc)
    return nc


def _rope_tables(position_ids):
    inv = (
        1.0
        / (10000.0 ** (np.arange(0, HD, 2, dtype=np.float32) / np.float32(HD)))
    ).astype(np.float32)
    t = np.arange(S, dtype=np.float32)
    freqs = (t[:, None] * inv[None, :]).astype(np.float32)
    emb = np.concatenate([freqs, freqs], axis=-1)
    cosf = np.cos(emb).astype(np.float32)
    sinf = np.sin(emb).astype(np.float32)
    pos = np.asarray(position_ids)
    cos_b = cosf[pos]  # [B, S, HD]
    sin_b = sinf[pos]
    sinS = sin_b.copy()
    sinS[:, :, : HD // 2] *= -1.0
    return cos_b, sinS


def kernel(x, position_ids, Wq, Wk, Wv, Wo):
    x = np.asarray(x, dtype=np.float32)
    Wq, Wk, Wv, Wo = (np.asarray(w, dtype=np.float32) for w in (Wq, Wk, Wv, Wo))
    cos_b, sinS_b = _rope_tables(position_ids)
    maskT = np.triu(np.ones((128, 128), dtype=np.float32))  # keep k<=q
    blocks = []
    for d in range(4):
        row = [np.zeros((128, 128), np.float32)] * d + [maskT] + [
            np.ones((128, 128), np.float32)
        ] * (3 - d)
        blocks.append(np.concatenate(row, axis=1))
    mask512 = np.concatenate(blocks, axis=1)  # [128, 4*512]
    ident = np.eye(128, dtype=np.float32)

    in_maps = []
    for c in range(8):
        b, hg = c // 4, (c % 4) * HPC
        rs = slice(hg * HD, hg * HD + DPC)
        in_maps.append(
            {
                "xT": np.ascontiguousarray(x[b].T),
                "wqT": np.ascontiguousarray(Wq[rs, :].T),
                "wkT": np.ascontiguousarray(Wk[rs, :].T),
                "wvT": np.ascontiguousarray(Wv[rs, :].T),
                "woT": np.ascontiguousarray(Wo[:, rs].T),
                "cos": np.ascontiguousarray(cos_b[b]),
                "sinS": np.ascontiguousarray(sinS_b[b]),
                "maskT": maskT,
                "ident": ident,
                "onesd": np.ones((128, 64), dtype=np.float32),
                "mask512": mask512,
            }
        )

    nc = build_nc()
    res = run_bass_kernel_spmd(nc, in_maps, list(range(8))).results

    y = np.zeros((B, S, H), dtype=np.float32)
    k = np.zeros((B, NH, S, HD), dtype=np.float32)
    v = np.zeros((B, NH, S, HD), dtype=np.float32)
    for c in range(8):
        b, hg = c // 4, (c % 4) * HPC
        y[b] += res[c]["y"]
        k[b, hg : hg + HPC] = (
            res[c]["ko"].reshape(S, HPC, HD).transpose(1, 0, 2)
        )
        v[b, hg : hg + HPC] = (
            res[c]["vo"].reshape(S, HPC, HD).transpose(1, 0, 2)
        )
    return y, k, v
